# revision 19
# baseline (speedup 1.0000x reference)
"""Trainium2 Bass kernel for nn_Attention_43868795961547 (sparse_attention).

Reference computation per batch item (8 items, data-parallel over 8 cores):
  x  = LN(img[b]) @ w_qkv -> q,k,v (8 heads x 64)          [1024 tokens]
  kt,vt from LN(tab[b]) @ w_tab_qkv appended as key/value position 1024
  out = softmax(q k^T / 8) @ v ; out @ w_out + b_out        -> [1024, 512]

Strategy (per core):
  - ln_w/ln_b folded into the qkv weights on host (w' = ln_w[:,None]*w,
    bias row = ln_b @ w); all matmul operands bf16 (PSUM stays fp32), so
    LDWEIGHTS gets fast-weight-load and K<128 matmuls run full rate.
  - LN stats token-major (bn_stats); normalized x cast to bf16, transposed
    via PE into xnT [512f x 1024t]; PSUM->SBUF copy on ACT.
  - qT,kT feature-major [512 x 1024] with per-feature bias added on the
    DVE copy out of PSUM; v token-major [1024 x 512] with per-head
    interleaved ones column (65-col groups) so attn@v accumulates softmax
    denominators for free (M=65 matmuls); v bias row added on the copy.
  - dots^T[kpos, q] per head via K=64 bf16 matmuls, two heads packed into
    disjoint PE row groups; exp on ACT straight out of PSUM (scale=1/8).
  - attn@v: out^T[65, q] accumulation over 8 kpos chunks + K=1 tab matmul.
  - denominators reciprocal'd on DVE, broadcast across partitions via
    GPSIMD partition_broadcast (no DRAM bounce), normalization applied to
    out^T, final matmul lhsT=out^T bf16, rhs=w_out bf16, bias added.
"""

import numpy as np
import ml_dtypes

import concourse.bass as bass
import concourse.mybir as mybir
import concourse.tile as tile
from concourse import bacc
from concourse import bass_utils
from concourse.masks import make_identity

F32 = mybir.dt.float32
BF16 = mybir.dt.bfloat16
NPBF16 = np.dtype(ml_dtypes.bfloat16)

N_CORES = 8
NTOK = 1024  # img tokens per batch item
DIM = 512
HEADS = 8
DHEAD = 64
INNER = 512
SCALE = DHEAD ** -0.5  # 0.125
EPS = 1e-5

NT = NTOK // 128   # 8 token tiles
NC_ = DIM // 128   # 4 feature chunks
NQB = 2            # q blocks of 512
QB = 512

import os
PHASE_LIMIT = int(os.environ.get("KERNEL_PHASE_LIMIT", "99"))


def build_program():
    nc = bacc.Bacc(
        "TRN2",
        target_bir_lowering=False,
        debug=False,
        enable_asserts=False,
        num_devices=N_CORES,
    )

    img = nc.dram_tensor("img_s", [NTOK, DIM], F32, kind="ExternalInput").ap()
    tab = nc.dram_tensor("tab_s", [1, DIM], F32, kind="ExternalInput").ap()
    # ln_w folded into all weights on host; biases = ln_b @ w
    w_qkv = nc.dram_tensor("w_qkv", [DIM, 3 * INNER], BF16, kind="ExternalInput").ap()
    # only k,v columns of w_tab_qkv (cols 512:1536) are used
    w_tab = nc.dram_tensor("w_tab", [DIM, 2 * INNER], BF16, kind="ExternalInput").ap()
    w_out = nc.dram_tensor("w_out", [INNER, DIM], BF16, kind="ExternalInput").ap()
    b_out = nc.dram_tensor("b_out", [1, DIM], F32, kind="ExternalInput").ap()
    # q,k bias feature-major: bqk[p, m] = (ln_b @ w_qkv)[m*128 + p], m in 0..7
    bqk = nc.dram_tensor("bqk", [128, 8], F32, kind="ExternalInput").ap()
    bv = nc.dram_tensor("bv", [1, INNER], F32, kind="ExternalInput").ap()
    btabk = nc.dram_tensor("btabk", [128, 4], F32, kind="ExternalInput").ap()
    btabv = nc.dram_tensor("btabv", [1, INNER], F32, kind="ExternalInput").ap()
    out_d = nc.dram_tensor("out_s", [NTOK, DIM], F32, kind="ExternalOutput").ap()

    with tile.TileContext(nc) as tc:
        kernel_body(tc, img, tab, w_qkv, w_tab, w_out, b_out,
                    bqk, bv, btabk, btabv, out_d)

    nc.compile()
    return nc


def kernel_body(tc, img, tab, w_qkv, w_tab, w_out, b_out,
                bqk, bv, btabk, btabv, out_d):
    nc = tc.nc
    AF = mybir.ActivationFunctionType
    OP = mybir.AluOpType

    import contextlib
    ctx = contextlib.ExitStack()
    with ctx:
        # ---------------- persistent pools ----------------
        const_p = ctx.enter_context(tc.tile_pool(name="const", bufs=1))
        qkT_p = ctx.enter_context(tc.tile_pool(name="qkT", bufs=1))
        v_p = ctx.enter_context(tc.tile_pool(name="vp", bufs=1))
        outuT_p = ctx.enter_context(tc.tile_pool(name="outuT", bufs=1))
        small_p = ctx.enter_context(tc.tile_pool(name="smallp", bufs=1))
        wo_p = ctx.enter_context(tc.tile_pool(name="wop", bufs=1))

        # psum pools (8 banks total):
        #  big:  2 tags x [128,1024] f32 (2 banks each) = 4 banks (dots/qkv/v)
        #  half: 2 tags x [128,512] f32 = 2 banks (transposes via bf16 bitcast,
        #        tab-dots, final proj) -- double-buffered
        #  o:    2 tags x [65,512] = 2 banks (attn@v out, tab k/v)
        psum_big = ctx.enter_context(tc.tile_pool(name="psbig", bufs=1, space="PSUM"))
        psum_o = ctx.enter_context(tc.tile_pool(name="pso", bufs=1, space="PSUM"))
        psum_h = ctx.enter_context(tc.tile_pool(name="psh", bufs=1, space="PSUM"))
        bigctr = [0]

        def big_tile(name):
            t = psum_big.tile([128, QB], F32, name=name,
                              tag=f"big{bigctr[0] % 4}")
            bigctr[0] += 1
            return t

        halfctr = [0]

        def half_tile(name):
            t = psum_h.tile([128, QB], F32, name=name,
                            tag=f"h{halfctr[0] % 2}")
            halfctr[0] += 1
            return t

        octr = [0]

        def o_tile(shape, name):
            t = psum_o.tile(shape, F32, name=name, tag=f"o{octr[0] % 2}")
            octr[0] += 1
            return t

        # ---------------- constants ----------------
        ident = const_p.tile([128, 128], F32, name="ident")
        make_identity(nc, ident)
        ones128 = const_p.tile([128, 128], BF16, name="ones128")
        nc.vector.memset(ones128, 1.0)

        eps_t = const_p.tile([128, 1], F32, name="eps_t")
        nc.vector.memset(eps_t, EPS)

        bout_bc = const_p.tile([128, DIM], F32, name="bout_bc")
        nc.gpsimd.dma_start(out=bout_bc, in_=b_out.to_broadcast([128, DIM]))
        bv_bc = const_p.tile([128, DIM], F32, name="bv_bc")
        nc.gpsimd.dma_start(out=bv_bc, in_=bv.to_broadcast([128, DIM]))
        bqk_c = const_p.tile([128, 8], F32, name="bqk_c")
        nc.gpsimd.dma_start(out=bqk_c, in_=bqk)
        btabk_c = const_p.tile([128, 4], F32, name="btabk_c")
        nc.gpsimd.dma_start(out=btabk_c, in_=btabk)
        btabv_r = const_p.tile([1, INNER], F32, name="btabv_r")
        nc.gpsimd.dma_start(out=btabv_r, in_=btabv)

        ones8 = const_p.tile([128, 8], BF16, name="ones8")
        nc.vector.memset(ones8, 1.0)

        # w_out chunks, persistent (used in final phase; DMA'd after wq/wt)
        wo = [wo_p.tile([128, DIM], BF16, name=f"wo{c}", tag=f"wo{c}")
              for c in range(NC_)]

        # ---------------- persistent activations ----------------
        qT = [qkT_p.tile([128, NTOK], BF16, name=f"qT{c}", tag=f"qT{c}") for c in range(NC_)]
        kT = [qkT_p.tile([128, NTOK], BF16, name=f"kT{c}", tag=f"kT{c}") for c in range(NC_)]
        # v token-major with interleaved ones column per head: 8 x (64+1) = 520
        v_sb = [v_p.tile([128, 520], BF16, name=f"v{t}", tag=f"v{t}") for t in range(NT)]
        # unnormalized out^T chunks [128, 1024]
        outuT = [outuT_p.tile([128, NTOK], BF16, name=f"ouT{c}", tag=f"ouT{c}") for c in range(NC_)]

        k_tT = small_p.tile([128, NC_], BF16, name="k_tT")  # col c = feat chunk c of tab key
        # v_t replicated to all partitions so K=1 tab matmuls can use any row strip
        v_t_aug = small_p.tile([128, 520], BF16, name="v_t_aug")
        # denominator collectors: D[g] row 32*j = head 4g+j (quadrant-aligned)
        dcol = [small_p.tile([128, NTOK], F32, name=f"dcol{g}") for g in range(2)]
        rcol = [small_p.tile([128, NTOK], F32, name=f"rcol{g}") for g in range(2)]
        rcolb = [small_p.tile([128, NTOK], BF16, name=f"rcolb{g}") for g in range(2)]
        rscratch = small_p.tile([128, NTOK], F32, name="rscratch")
        for g in range(2):
            nc.vector.memset(dcol[g], 1.0)
        # tab-dot exp rows: [qb][g] holds heads 4g..4g+3 at partitions {0,32,64,96}
        tabexp = small_p.tile([128, 2 * NTOK], BF16, name="tabexp")

        # ---------------- phase 1+2: LN, transpose, qkv (scoped weights) ----------------
        with tc.tile_pool(name="wqkv", bufs=1) as wq_p, \
             tc.tile_pool(name="xnt", bufs=1) as xnT_p, \
             tc.tile_pool(name="lnt", bufs=2) as ln_p:

            # img tiles first: LN starts immediately; weights stream behind
            x_ts = []
            for t in range(NT):
                x_t = ln_p.tile([128, DIM], F32, name="x_t", tag="x_t", bufs=8)
                nc.sync.dma_start(out=x_t, in_=img[t * 128:(t + 1) * 128, :])
                x_ts.append(x_t)

            wq = []
            for c in range(NC_):
                t = wq_p.tile([128, 3 * INNER], BF16, name=f"wq{c}", tag=f"wq{c}")
                nc.sync.dma_start(out=t, in_=w_qkv[c * 128:(c + 1) * 128, :])
                wq.append(t)
            wt = []
            for c in range(NC_):
                t = wq_p.tile([128, 2 * INNER], BF16, name=f"wt{c}", tag=f"wt{c}")
                nc.sync.dma_start(out=t, in_=w_tab[c * 128:(c + 1) * 128, :])
                wt.append(t)

            for c in range(NC_):
                nc.sync.dma_start(out=wo[c], in_=w_out[c * 128:(c + 1) * 128, :])

            xnT = [xnT_p.tile([128, NTOK], BF16, name=f"xnT{c}", tag=f"xnT{c}")
                   for c in range(NC_)]

            # --- img layernorm + transpose ---
            for t in range(NT):
                x_t = x_ts[t]

                stats = ln_p.tile([128, 6], F32, name="stats", tag="stats")
                nc.vector.bn_stats(out=stats, in_=x_t)
                mv = ln_p.tile([128, 2], F32, name="mv", tag="mv")
                nc.vector.bn_aggr(out=mv, in_=stats)

                sd = ln_p.tile([128, 1], F32, name="sd", tag="sd")
                nc.scalar.activation(out=sd, in_=mv[:, 1:2], func=AF.Sqrt,
                                     bias=eps_t, scale=1.0)
                rstd = ln_p.tile([128, 1], F32, name="rstd", tag="rstd")
                nc.vector.reciprocal(out=rstd, in_=sd)

                xn_t = ln_p.tile([128, DIM], F32, name="xn_t", tag="xn_t", bufs=4)
                nc.vector.tensor_scalar(out=xn_t, in0=x_t,
                                        scalar1=mv[:, 0:1], scalar2=rstd,
                                        op0=OP.subtract, op1=OP.mult)

                for c in range(NC_):
                    phb = half_tile("pt")
                    pt = phb[:, 0:128]
                    nc.tensor.transpose(out=pt, in_=xn_t[:, c * 128:(c + 1) * 128],
                                        identity=ident)
                    nc.scalar.copy(out=xnT[c][:, t * 128:(t + 1) * 128], in_=pt)

            # --- tab layernorm (1 row) ---
            tb = ln_p.tile([1, DIM], F32, name="tb", tag="tb", bufs=1)
            nc.sync.dma_start(out=tb, in_=tab)
            tstats = ln_p.tile([1, 6], F32, name="tstats", tag="tstats")
            nc.vector.bn_stats(out=tstats, in_=tb)
            tmv = ln_p.tile([1, 2], F32, name="tmv", tag="tmv")
            nc.vector.bn_aggr(out=tmv, in_=tstats)
            tsd = ln_p.tile([1, 1], F32, name="tsd", tag="tsd")
            nc.scalar.activation(out=tsd, in_=tmv[:, 1:2], func=AF.Sqrt,
                                 bias=eps_t[0:1], scale=1.0)
            trstd = ln_p.tile([1, 1], F32, name="trstd", tag="trstd")
            nc.vector.reciprocal(out=trstd, in_=tsd)
            tn = ln_p.tile([1, DIM], F32, name="tn", tag="tn", bufs=1)
            nc.vector.tensor_scalar(out=tn, in0=tb, scalar1=tmv[:, 0:1],
                                    scalar2=trstd, op0=OP.subtract, op1=OP.mult)

            # transpose tn -> tnT [512,1] as 4 chunks [128,1]
            tnT = ln_p.tile([128, NC_], BF16, name="tnT", tag="tnT", bufs=1)
            for c in range(NC_):
                phb = half_tile("ptn")
                pt = phb[:, 0:1]
                nc.tensor.transpose(out=pt, in_=tn[0:1, c * 128:(c + 1) * 128],
                                    identity=ident[0:1, 0:1])
                nc.vector.tensor_copy(out=tnT[:, c:c + 1], in_=pt)

            # --- qT, kT: feature-major qkv (+ bias on the copy) ---
            # order: head-pair 0's q+k first so dots can start early
            for m in (0, 4, 1, 5, 2, 6, 3, 7):
                dst = qT[m] if m < 4 else kT[m - 4]
                for qb in range(NQB):
                    ps = big_tile("psqk")
                    for kc in range(NC_):
                        nc.tensor.matmul(
                            ps,
                            lhsT=(wq[kc][:, m * 128:(m + 1) * 128]),
                            rhs=(xnT[kc][:, qb * QB:(qb + 1) * QB]),
                            start=(kc == 0), stop=(kc == NC_ - 1))
                    nc.vector.tensor_scalar(
                        out=dst[:, qb * QB:(qb + 1) * QB], in0=ps,
                        scalar1=bqk_c[:, m:m + 1], scalar2=None, op0=OP.add)

            # --- v token-major (+ bias row + ones interleave) ---
            for t in range(NT):
                pv = big_tile("psv")
                for kc in range(NC_):
                    nc.tensor.matmul(
                        pv,
                        lhsT=(xnT[kc][:, t * 128:(t + 1) * 128]),
                        rhs=(wq[kc][:, 2 * INNER:3 * INNER]),
                        start=(kc == 0), stop=(kc == NC_ - 1))
                vdst = v_sb[t].rearrange("p (h s) -> p h s", s=65)
                nc.vector.tensor_tensor(
                    out=vdst[:, :, 0:64],
                    in0=pv.rearrange("p (h d) -> p h d", d=64),
                    in1=bv_bc.rearrange("p (h d) -> p h d", d=64),
                    op=OP.add)
                nc.vector.tensor_copy(
                    out=vdst[:, :, 64:65],
                    in_=ones8.rearrange("p (h o) -> p h o", o=1))

            # --- tab k/v ---
            for c in range(NC_):
                ps = o_tile([128, 1], "pskt")
                for kc in range(NC_):
                    nc.tensor.matmul(
                        ps,
                        lhsT=wt[kc][:, c * 128:(c + 1) * 128],
                        rhs=tnT[:, kc:kc + 1],
                        start=(kc == 0), stop=(kc == NC_ - 1))
                nc.vector.tensor_scalar(
                    out=k_tT[:, c:c + 1], in0=ps,
                    scalar1=btabk_c[:, c:c + 1], scalar2=None, op0=OP.add)
            ps_vt = o_tile([1, INNER], "psvt")
            for kc in range(NC_):
                nc.tensor.matmul(
                    ps_vt,
                    lhsT=(tnT[:, kc:kc + 1]),
                    rhs=(wt[kc][:, INNER:2 * INNER]),
                    start=(kc == 0), stop=(kc == NC_ - 1))
            vta = v_t_aug[0:1, :].rearrange("p (h s) -> p h s", s=65)
            nc.vector.tensor_tensor(
                out=vta[:, :, 0:64],
                in0=ps_vt.rearrange("p (h d) -> p h d", d=64),
                in1=btabv_r.rearrange("p (h d) -> p h d", d=64),
                op=OP.add)
            nc.vector.tensor_copy(
                out=vta[:, :, 64:65],
                in_=ones8[0:1, :].rearrange("p (h o) -> p h o", o=1))
            nc.gpsimd.partition_broadcast(out_ap=v_t_aug, in_ap=v_t_aug[0:1, :])

        if PHASE_LIMIT <= 2:
            with tc.tile_pool(name="dbg", bufs=2) as dbg_p:
                for t in range(NT):
                    fo = dbg_p.tile([128, DIM], F32, name="dbg_fo", tag="fo")
                    nc.vector.tensor_copy(out=fo, in_=qT[t % 4][:, 0:DIM])
                    nc.sync.dma_start(out=out_d[t * 128:(t + 1) * 128, :], in_=fo)
            return

        # ---------------- phase 3: tab dots + exp ----------------
        # head h key chunk: k_tT feat rows 64h..64h+63 live at
        # partitions (h%2)*64.. of column h//2
        for qb in range(NQB):
            for g in range(2):  # head groups of 4
                ps = half_tile("pstd")
                nc.vector.memset(ps, 0.0)
                for j in range(4):
                    h = 4 * g + j
                    hb = (h % 2) * 64
                    nc.tensor.matmul(
                        ps[32 * j:32 * j + 1, :],
                        lhsT=k_tT[hb:hb + 64, h // 2:h // 2 + 1],
                        rhs=qT[h // 2][hb:hb + 64, qb * QB:(qb + 1) * QB],
                        start=True, stop=True,
                        tile_position=(hb, 32 * j))
                nc.scalar.activation(
                    out=tabexp[0:97, (2 * qb + g) * QB:(2 * qb + g + 1) * QB],
                    in_=ps[0:97, :], func=AF.Exp, scale=SCALE)

        if PHASE_LIMIT <= 3:
            with tc.tile_pool(name="dbg", bufs=2) as dbg_p:
                for t in range(NT):
                    fo = dbg_p.tile([128, DIM], F32, name="dbg_fo", tag="fo")
                    nc.vector.tensor_copy(out=fo, in_=tabexp[:, 0:DIM])
                    nc.sync.dma_start(out=out_d[t * 128:(t + 1) * 128, :], in_=fo)
            return

        # ---------------- phase 4+5: attention, normalize, final (qb-outer) ----
        # Per q-block: all heads' dots/exp/attn@v, then recip+normalize+final
        # for that q-block while the next q-block's attention runs.
        ae_p = ctx.enter_context(tc.tile_pool(name="aep", bufs=2))
        fo_p = ctx.enter_context(tc.tile_pool(name="fout", bufs=2))

        for qb in range(NQB):
            for hp in range(4):
                h0, h1 = 2 * hp, 2 * hp + 1
                ae0 = ae_p.tile([128, 8 * QB], BF16, name="ae0", tag="ae0")
                ae1 = ae_p.tile([128, 8 * QB], BF16, name="ae1", tag="ae1")
                # dots^T + exp: one single-bank psum tile per (kt, head)
                # MM so the 4-deep rotation decouples PE from exp latency
                for kt in range(NT):
                    for hh, ae in ((0, ae0), (1, ae1)):
                        hb = hh * 64
                        ps = big_tile("psd")
                        nc.tensor.matmul(
                            ps,
                            lhsT=kT[hp][hb:hb + 64, kt * 128:(kt + 1) * 128],
                            rhs=qT[hp][hb:hb + 64, qb * QB:(qb + 1) * QB],
                            start=True, stop=True)
                        nc.scalar.activation(
                            out=ae[:, kt * QB:(kt + 1) * QB],
                            in_=ps, func=AF.Exp, scale=SCALE)

                # attn@v with denominator row (M=65)
                for hh, ae in ((0, ae0), (1, ae1)):
                    h = 2 * hp + hh
                    po = o_tile([65, QB], "po")
                    for kt in range(NT):
                        nc.tensor.matmul(
                            po,
                            lhsT=v_sb[kt][:, 65 * h:65 * h + 65],
                            rhs=ae[:, kt * QB:(kt + 1) * QB],
                            start=(kt == 0), stop=False)
                    # tab token contribution (K=1, row strip matching tabexp row)
                    g, j = h // 4, h % 4
                    nc.tensor.matmul(
                        po,
                        lhsT=v_t_aug[32 * j:32 * j + 1, 65 * h:65 * h + 65],
                        rhs=tabexp[32 * j:32 * j + 1,
                                   (2 * qb + g) * QB:(2 * qb + g + 1) * QB],
                        start=False, stop=True,
                        tile_position=(32 * j, 0))
                    # cross-quadrant moves must stay on DVE (bank->quadrant
                    # routing is a DVE feature; ACT semantics unknown)
                    if hh == 0:
                        nc.vector.tensor_copy(
                            out=outuT[hp][0:64, qb * QB:(qb + 1) * QB],
                            in_=po[0:64, :])
                    else:
                        nc.vector.tensor_copy(
                            out=outuT[hp][64:128, qb * QB:(qb + 1) * QB],
                            in_=po[0:64, :])
                    nc.vector.tensor_copy(
                        out=dcol[h // 4][32 * (h % 4):32 * (h % 4) + 1,
                                         qb * QB:(qb + 1) * QB],
                        in_=po[64:65, :])

                # after the 2nd/4th pair, this head-group's denominators are
                # complete: recip -> PE K=1 broadcast matmul -> normalize
                if hp in (1, 3):
                    g = hp // 2
                    qs_ = slice(qb * QB, (qb + 1) * QB)
                    nc.vector.reciprocal_approx_accurate(
                        out=rcol[g][0:97, qs_], in_=dcol[g][0:97, qs_],
                        scratch=rscratch[0:97, 0:QB])
                    nc.vector.tensor_copy(out=rcolb[g][0:97, qs_],
                                          in_=rcol[g][0:97, qs_])
                    # one psum bc tile per pair: rows 0-63 even head, 64-127 odd
                    for pp in range(2):
                        hp_ = 2 * g + pp
                        h0_, h1_ = 2 * hp_, 2 * hp_ + 1
                        pbc = o_tile([128, QB], "pbc")
                        for hh_, h_ in ((0, h0_), (1, h1_)):
                            j_ = 32 * (h_ % 4)
                            nc.tensor.matmul(
                                pbc[64 * hh_:64 * hh_ + 64, :],
                                lhsT=ones128[j_:j_ + 1, 0:64],
                                rhs=rcolb[g][j_:j_ + 1, qs_],
                                start=True, stop=True,
                                tile_position=(j_, 64 * hh_))
                        nc.vector.tensor_tensor(
                            out=outuT[hp_][:, qs_],
                            in0=outuT[hp_][:, qs_],
                            in1=pbc, op=OP.mult)

            # ---- normalize handled per head-group inside the hp loop ----
            qs = slice(qb * QB, (qb + 1) * QB)
            for t in range(4 * qb, 4 * qb + 4):
                pf = half_tile("pf")
                for c in range(NC_):
                    nc.tensor.matmul(
                        pf,
                        lhsT=outuT[c][:, t * 128:(t + 1) * 128],
                        rhs=wo[c],
                        start=(c == 0), stop=(c == NC_ - 1))
                fo = fo_p.tile([128, DIM], F32, name="fo", tag="fo")
                nc.vector.tensor_tensor(out=fo, in0=pf, in1=bout_bc, op=OP.add)
                nc.sync.dma_start(out=out_d[t * 128:(t + 1) * 128, :], in_=fo)


_CACHED_NC = None


def _preprocess(inputs):
    img = np.ascontiguousarray(np.asarray(inputs["img"], dtype=np.float32))
    tab = np.ascontiguousarray(np.asarray(inputs["tab"], dtype=np.float32))
    w_qkv = np.asarray(inputs["w_qkv"], dtype=np.float32)
    w_tab_qkv = np.asarray(inputs["w_tab_qkv"], dtype=np.float32)
    w_out = np.asarray(inputs["w_out"], dtype=np.float32)
    b_out = np.asarray(inputs["b_out"], dtype=np.float32).reshape(1, DIM)
    ln_w = np.asarray(inputs["ln_w"], dtype=np.float32).reshape(DIM)
    ln_b = np.asarray(inputs["ln_b"], dtype=np.float32).reshape(DIM)

    w_tab_kv = w_tab_qkv[:, INNER:3 * INNER]
    # fold ln scale/shift into the weights (LN is linear before the matmul)
    w_qkv_eff = (ln_w[:, None] * w_qkv).astype(NPBF16)
    w_tab_eff = (ln_w[:, None] * w_tab_kv).astype(NPBF16)
    b_qkv = ln_b @ w_qkv          # [1536]
    b_tab = ln_b @ w_tab_kv       # [1024]

    shared = {
        "w_qkv": np.ascontiguousarray(w_qkv_eff),
        "w_tab": np.ascontiguousarray(w_tab_eff),
        "w_out": np.ascontiguousarray(w_out.astype(NPBF16)),
        "b_out": b_out,
        "bqk": np.ascontiguousarray(b_qkv[:2 * INNER].reshape(8, 128).T
                                    .astype(np.float32)),
        "bv": np.ascontiguousarray(b_qkv[2 * INNER:].reshape(1, INNER)
                                   .astype(np.float32)),
        "btabk": np.ascontiguousarray(b_tab[:INNER].reshape(4, 128).T
                                      .astype(np.float32)),
        "btabv": np.ascontiguousarray(b_tab[INNER:].reshape(1, INNER)
                                      .astype(np.float32)),
    }
    in_maps = []
    for b in range(N_CORES):
        m = {"img_s": np.ascontiguousarray(img[b]),
             "tab_s": np.ascontiguousarray(tab[b])}
        m.update(shared)
        in_maps.append(m)
    return in_maps


def make_in_maps(inputs):
    return _preprocess(inputs)


def kernel(**inputs):
    global _CACHED_NC
    if _CACHED_NC is None:
        _CACHED_NC = build_program()
    nc = _CACHED_NC

    in_maps = _preprocess(inputs)
    res = bass_utils.run_bass_kernel_spmd(nc, in_maps, core_ids=list(range(N_CORES)))
    out = np.stack([res.results[c]["out_s"] for c in range(N_CORES)], axis=0)
    return out.astype(np.float32)


if __name__ == "__main__":
    d = np.load("/root/problem/ref_data.npz")
    ins = {k: d[k] for k in ("img", "tab", "w_qkv", "w_tab_qkv", "w_out",
                             "b_out", "ln_w", "ln_b")}
    actual = kernel(**ins)
    expected = d["expected"]
    err = np.abs(actual - expected).max()
    rel = err / np.abs(expected).max()
    print("absmax err:", err, "rel:", rel)


# revision 21
# speedup vs baseline: 1.1209x; 1.1209x over previous
"""Trainium2 Bass kernel for nn_Attention_43868795961547 (sparse_attention).

Reference computation per batch item (8 items, data-parallel over 8 cores):
  x  = LN(img[b]) @ w_qkv -> q,k,v (8 heads x 64)          [1024 tokens]
  kt,vt from LN(tab[b]) @ w_tab_qkv appended as key/value position 1024
  out = softmax(q k^T / 8) @ v ; out @ w_out + b_out        -> [1024, 512]

Strategy (per core):
  - ln_w/ln_b folded into the qkv weights on host (w' = ln_w[:,None]*w,
    bias row = ln_b @ w); all matmul operands bf16 (PSUM stays fp32), so
    LDWEIGHTS gets fast-weight-load and K<128 matmuls run full rate.
  - LN stats token-major (bn_stats); normalized x cast to bf16, transposed
    via PE into xnT [512f x 1024t]; PSUM->SBUF copy on ACT.
  - qT,kT feature-major [512 x 1024] with per-feature bias added on the
    DVE copy out of PSUM; v token-major [1024 x 512] with per-head
    interleaved ones column (65-col groups) so attn@v accumulates softmax
    denominators for free (M=65 matmuls); v bias row added on the copy.
  - dots^T[kpos, q] per head via K=64 bf16 matmuls, two heads packed into
    disjoint PE row groups; exp on ACT straight out of PSUM (scale=1/8).
  - attn@v: out^T[65, q] accumulation over 8 kpos chunks + K=1 tab matmul.
  - denominators reciprocal'd on DVE, broadcast across partitions via
    GPSIMD partition_broadcast (no DRAM bounce), normalization applied to
    out^T, final matmul lhsT=out^T bf16, rhs=w_out bf16, bias added.
"""

import numpy as np
import ml_dtypes

import concourse.bass as bass
import concourse.mybir as mybir
import concourse.tile as tile
from concourse import bacc
from concourse import bass_utils
from concourse.masks import make_identity

F32 = mybir.dt.float32
BF16 = mybir.dt.bfloat16
NPBF16 = np.dtype(ml_dtypes.bfloat16)

N_CORES = 8
NTOK = 1024  # img tokens per batch item
DIM = 512
HEADS = 8
DHEAD = 64
INNER = 512
SCALE = DHEAD ** -0.5  # 0.125
EPS = 1e-5

NT = NTOK // 128   # 8 token tiles
NC_ = DIM // 128   # 4 feature chunks
NQB = 2            # q blocks of 512
QB = 512

import os
PHASE_LIMIT = int(os.environ.get("KERNEL_PHASE_LIMIT", "99"))


def build_program():
    nc = bacc.Bacc(
        "TRN2",
        target_bir_lowering=False,
        debug=False,
        enable_asserts=False,
        num_devices=N_CORES,
    )

    img = nc.dram_tensor("img_s", [NTOK, DIM], F32, kind="ExternalInput").ap()
    tab = nc.dram_tensor("tab_s", [1, DIM], F32, kind="ExternalInput").ap()
    # ln_w folded into all weights on host; biases = ln_b @ w
    w_qkv = nc.dram_tensor("w_qkv", [DIM, 3 * INNER], BF16, kind="ExternalInput").ap()
    # only k,v columns of w_tab_qkv (cols 512:1536) are used
    w_tab = nc.dram_tensor("w_tab", [DIM, 2 * INNER], BF16, kind="ExternalInput").ap()
    w_out = nc.dram_tensor("w_out", [INNER, DIM], BF16, kind="ExternalInput").ap()
    b_out = nc.dram_tensor("b_out", [1, DIM], F32, kind="ExternalInput").ap()
    # q,k bias feature-major: bqk[p, m] = (ln_b @ w_qkv)[m*128 + p], m in 0..7
    bqk = nc.dram_tensor("bqk", [128, 8], F32, kind="ExternalInput").ap()
    bv = nc.dram_tensor("bv", [1, INNER], F32, kind="ExternalInput").ap()
    btabk = nc.dram_tensor("btabk", [128, 4], F32, kind="ExternalInput").ap()
    btabv = nc.dram_tensor("btabv", [1, INNER], F32, kind="ExternalInput").ap()
    out_d = nc.dram_tensor("out_s", [NTOK, DIM], F32, kind="ExternalOutput").ap()

    with tile.TileContext(nc) as tc:
        kernel_body(tc, img, tab, w_qkv, w_tab, w_out, b_out,
                    bqk, bv, btabk, btabv, out_d)

    nc.compile()
    return nc


def kernel_body(tc, img, tab, w_qkv, w_tab, w_out, b_out,
                bqk, bv, btabk, btabv, out_d):
    nc = tc.nc
    AF = mybir.ActivationFunctionType
    OP = mybir.AluOpType

    import contextlib
    ctx = contextlib.ExitStack()
    with ctx:
        # ---------------- persistent pools ----------------
        const_p = ctx.enter_context(tc.tile_pool(name="const", bufs=1))
        qkT_p = ctx.enter_context(tc.tile_pool(name="qkT", bufs=1))
        v_p = ctx.enter_context(tc.tile_pool(name="vp", bufs=1))
        outuT_p = ctx.enter_context(tc.tile_pool(name="outuT", bufs=1))
        small_p = ctx.enter_context(tc.tile_pool(name="smallp", bufs=1))
        wo_p = ctx.enter_context(tc.tile_pool(name="wop", bufs=1))

        # psum pools (8 banks total):
        #  big:  2 tags x [128,1024] f32 (2 banks each) = 4 banks (dots/qkv/v)
        #  half: 2 tags x [128,512] f32 = 2 banks (transposes via bf16 bitcast,
        #        tab-dots, final proj) -- double-buffered
        #  o:    2 tags x [65,512] = 2 banks (attn@v out, tab k/v)
        psum_big = ctx.enter_context(tc.tile_pool(name="psbig", bufs=1, space="PSUM"))
        psum_o = ctx.enter_context(tc.tile_pool(name="pso", bufs=1, space="PSUM"))
        psum_h = ctx.enter_context(tc.tile_pool(name="psh", bufs=1, space="PSUM"))
        bigctr = [0]

        def big_tile(name):
            t = psum_big.tile([128, 2 * QB], F32, name=name,
                              tag=f"big{bigctr[0] % 2}")
            bigctr[0] += 1
            return t

        halfctr = [0]

        def half_tile(name):
            t = psum_h.tile([128, QB], F32, name=name,
                            tag=f"h{halfctr[0] % 2}")
            halfctr[0] += 1
            return t

        octr = [0]

        def o_tile(shape, name):
            t = psum_o.tile(shape, F32, name=name, tag=f"o{octr[0] % 2}")
            octr[0] += 1
            return t

        # ---------------- constants ----------------
        ident = const_p.tile([128, 128], F32, name="ident")
        make_identity(nc, ident)
        ones128 = const_p.tile([128, 128], BF16, name="ones128")

        eps_t = const_p.tile([128, 1], F32, name="eps_t")
        nc.vector.memset(eps_t, EPS)

        bout_bc = const_p.tile([128, DIM], F32, name="bout_bc")
        nc.gpsimd.dma_start(out=bout_bc, in_=b_out.to_broadcast([128, DIM]))
        bv_bc = const_p.tile([128, DIM], F32, name="bv_bc")
        nc.gpsimd.dma_start(out=bv_bc, in_=bv.to_broadcast([128, DIM]))
        bqk_c = const_p.tile([128, 8], F32, name="bqk_c")
        nc.gpsimd.dma_start(out=bqk_c, in_=bqk)
        btabk_c = const_p.tile([128, 4], F32, name="btabk_c")
        nc.gpsimd.dma_start(out=btabk_c, in_=btabk)
        btabv_r = const_p.tile([1, INNER], F32, name="btabv_r")
        nc.gpsimd.dma_start(out=btabv_r, in_=btabv)

        ones8 = const_p.tile([128, 8], BF16, name="ones8")
        nc.vector.memset(ones8, 1.0)

        # w_out chunks, persistent (used in final phase; DMA'd after wq/wt)
        wo = [wo_p.tile([128, DIM], BF16, name=f"wo{c}", tag=f"wo{c}")
              for c in range(NC_)]

        # ---------------- persistent activations ----------------
        qT = [qkT_p.tile([128, NTOK], BF16, name=f"qT{c}", tag=f"qT{c}") for c in range(NC_)]
        kT = [qkT_p.tile([128, NTOK], BF16, name=f"kT{c}", tag=f"kT{c}") for c in range(NC_)]
        # v token-major with interleaved ones column per head: 8 x (64+1) = 520
        v_sb = [v_p.tile([128, 520], BF16, name=f"v{t}", tag=f"v{t}") for t in range(NT)]
        # unnormalized out^T chunks [128, 1024]
        outuT = [outuT_p.tile([128, NTOK], BF16, name=f"ouT{c}", tag=f"ouT{c}") for c in range(NC_)]

        k_tT = small_p.tile([128, NC_], BF16, name="k_tT")  # col c = feat chunk c of tab key
        # v_t replicated to all partitions so K=1 tab matmuls can use any row strip
        v_t_aug = small_p.tile([128, 520], BF16, name="v_t_aug")
        # denominator collectors: D[g] row 32*j = head 4g+j (quadrant-aligned)
        dcol = [small_p.tile([128, NTOK], F32, name=f"dcol{g}") for g in range(2)]
        rcol = [small_p.tile([128, NTOK], F32, name=f"rcol{g}") for g in range(2)]
        rcolb = [small_p.tile([128, NTOK], BF16, name=f"rcolb{g}") for g in range(2)]
        rscratch = small_p.tile([128, NTOK], F32, name="rscratch")
        # tab-dot exp rows: [qb][g] holds heads 4g..4g+3 at partitions {0,32,64,96}
        tabexp = small_p.tile([128, 2 * NTOK], BF16, name="tabexp")

        # ---------------- phase 1+2: LN, transpose, qkv (scoped weights) ----------------
        with tc.tile_pool(name="wqkv", bufs=1) as wq_p, \
             tc.tile_pool(name="xnt", bufs=1) as xnT_p, \
             tc.tile_pool(name="lnt", bufs=2) as ln_p:

            # img tiles first: LN starts immediately; weights stream behind
            x_ts = []
            for t in range(NT):
                x_t = ln_p.tile([128, DIM], F32, name="x_t", tag="x_t", bufs=8)
                nc.sync.dma_start(out=x_t, in_=img[t * 128:(t + 1) * 128, :])
                x_ts.append(x_t)

            wq = []
            for c in range(NC_):
                t = wq_p.tile([128, 3 * INNER], BF16, name=f"wq{c}", tag=f"wq{c}")
                nc.sync.dma_start(out=t, in_=w_qkv[c * 128:(c + 1) * 128, :])
                wq.append(t)
            wt = []
            for c in range(NC_):
                t = wq_p.tile([128, 2 * INNER], BF16, name=f"wt{c}", tag=f"wt{c}")
                nc.sync.dma_start(out=t, in_=w_tab[c * 128:(c + 1) * 128, :])
                wt.append(t)

            for c in range(NC_):
                nc.sync.dma_start(out=wo[c], in_=w_out[c * 128:(c + 1) * 128, :])

            xnT = [xnT_p.tile([128, NTOK], BF16, name=f"xnT{c}", tag=f"xnT{c}")
                   for c in range(NC_)]

            # --- img layernorm + transpose ---
            for t in range(NT):
                x_t = x_ts[t]

                stats = ln_p.tile([128, 6], F32, name="stats", tag="stats")
                nc.vector.bn_stats(out=stats, in_=x_t)
                mv = ln_p.tile([128, 2], F32, name="mv", tag="mv")
                nc.vector.bn_aggr(out=mv, in_=stats)

                sd = ln_p.tile([128, 1], F32, name="sd", tag="sd")
                nc.scalar.activation(out=sd, in_=mv[:, 1:2], func=AF.Sqrt,
                                     bias=eps_t, scale=1.0)
                rstd = ln_p.tile([128, 1], F32, name="rstd", tag="rstd")
                nc.vector.reciprocal(out=rstd, in_=sd)

                xn_t = ln_p.tile([128, DIM], F32, name="xn_t", tag="xn_t", bufs=4)
                nc.vector.tensor_scalar(out=xn_t, in0=x_t,
                                        scalar1=mv[:, 0:1], scalar2=rstd,
                                        op0=OP.subtract, op1=OP.mult)

                for c in range(NC_):
                    phb = half_tile("pt")
                    pt = phb[:, 0:128]
                    nc.tensor.transpose(out=pt, in_=xn_t[:, c * 128:(c + 1) * 128],
                                        identity=ident)
                    nc.scalar.copy(out=xnT[c][:, t * 128:(t + 1) * 128], in_=pt)

            # --- tab layernorm (1 row) ---
            tb = ln_p.tile([1, DIM], F32, name="tb", tag="tb", bufs=1)
            nc.sync.dma_start(out=tb, in_=tab)
            tstats = ln_p.tile([1, 6], F32, name="tstats", tag="tstats")
            nc.vector.bn_stats(out=tstats, in_=tb)
            tmv = ln_p.tile([1, 2], F32, name="tmv", tag="tmv")
            nc.vector.bn_aggr(out=tmv, in_=tstats)
            tsd = ln_p.tile([1, 1], F32, name="tsd", tag="tsd")
            nc.scalar.activation(out=tsd, in_=tmv[:, 1:2], func=AF.Sqrt,
                                 bias=eps_t[0:1], scale=1.0)
            trstd = ln_p.tile([1, 1], F32, name="trstd", tag="trstd")
            nc.vector.reciprocal(out=trstd, in_=tsd)
            tn = ln_p.tile([1, DIM], F32, name="tn", tag="tn", bufs=1)
            nc.vector.tensor_scalar(out=tn, in0=tb, scalar1=tmv[:, 0:1],
                                    scalar2=trstd, op0=OP.subtract, op1=OP.mult)

            # transpose tn -> tnT [512,1] as 4 chunks [128,1]
            tnT = ln_p.tile([128, NC_], BF16, name="tnT", tag="tnT", bufs=1)
            for c in range(NC_):
                phb = half_tile("ptn")
                pt = phb[:, 0:1]
                nc.tensor.transpose(out=pt, in_=tn[0:1, c * 128:(c + 1) * 128],
                                    identity=ident[0:1, 0:1])
                nc.vector.tensor_copy(out=tnT[:, c:c + 1], in_=pt)

            # --- qT, kT: feature-major qkv (+ bias on the copy) ---
            # order: head-pair 0's q+k first so dots can start early
            for m in (0, 4, 1, 5, 2, 6, 3, 7):
                dst = qT[m] if m < 4 else kT[m - 4]
                for qb in range(NQB):
                    ps = big_tile("psqk")
                    half = ps[:, 0:QB] if qb == 0 else ps[:, QB:2 * QB]
                    for kc in range(NC_):
                        nc.tensor.matmul(
                            half,
                            lhsT=(wq[kc][:, m * 128:(m + 1) * 128]),
                            rhs=(xnT[kc][:, qb * QB:(qb + 1) * QB]),
                            start=(kc == 0), stop=(kc == NC_ - 1))
                    nc.vector.tensor_scalar(
                        out=dst[:, qb * QB:(qb + 1) * QB], in0=half,
                        scalar1=bqk_c[:, m:m + 1], scalar2=None, op0=OP.add)

            # --- v token-major (+ bias row + ones interleave) ---
            for t in range(NT):
                ps = big_tile("psv")
                pv = ps[:, 0:QB]
                for kc in range(NC_):
                    nc.tensor.matmul(
                        pv,
                        lhsT=(xnT[kc][:, t * 128:(t + 1) * 128]),
                        rhs=(wq[kc][:, 2 * INNER:3 * INNER]),
                        start=(kc == 0), stop=(kc == NC_ - 1))
                vdst = v_sb[t].rearrange("p (h s) -> p h s", s=65)
                nc.vector.tensor_tensor(
                    out=vdst[:, :, 0:64],
                    in0=pv.rearrange("p (h d) -> p h d", d=64),
                    in1=bv_bc.rearrange("p (h d) -> p h d", d=64),
                    op=OP.add)
                nc.vector.tensor_copy(
                    out=vdst[:, :, 64:65],
                    in_=ones8.rearrange("p (h o) -> p h o", o=1))

            # --- tab k/v ---
            for c in range(NC_):
                ps = o_tile([128, 1], "pskt")
                for kc in range(NC_):
                    nc.tensor.matmul(
                        ps,
                        lhsT=wt[kc][:, c * 128:(c + 1) * 128],
                        rhs=tnT[:, kc:kc + 1],
                        start=(kc == 0), stop=(kc == NC_ - 1))
                nc.vector.tensor_scalar(
                    out=k_tT[:, c:c + 1], in0=ps,
                    scalar1=btabk_c[:, c:c + 1], scalar2=None, op0=OP.add)
            ps_vt = o_tile([1, INNER], "psvt")
            for kc in range(NC_):
                nc.tensor.matmul(
                    ps_vt,
                    lhsT=(tnT[:, kc:kc + 1]),
                    rhs=(wt[kc][:, INNER:2 * INNER]),
                    start=(kc == 0), stop=(kc == NC_ - 1))
            vta = v_t_aug[0:1, :].rearrange("p (h s) -> p h s", s=65)
            nc.vector.tensor_tensor(
                out=vta[:, :, 0:64],
                in0=ps_vt.rearrange("p (h d) -> p h d", d=64),
                in1=btabv_r.rearrange("p (h d) -> p h d", d=64),
                op=OP.add)
            nc.vector.tensor_copy(
                out=vta[:, :, 64:65],
                in_=ones8[0:1, :].rearrange("p (h o) -> p h o", o=1))
            nc.gpsimd.partition_broadcast(out_ap=v_t_aug, in_ap=v_t_aug[0:1, :])

        if PHASE_LIMIT <= 2:
            with tc.tile_pool(name="dbg", bufs=2) as dbg_p:
                for t in range(NT):
                    fo = dbg_p.tile([128, DIM], F32, name="dbg_fo", tag="fo")
                    nc.vector.tensor_copy(out=fo, in_=qT[t % 4][:, 0:DIM])
                    nc.sync.dma_start(out=out_d[t * 128:(t + 1) * 128, :], in_=fo)
            return

        # deferred phase-4 constant init (keeps DVE free during LN startup)
        nc.vector.memset(ones128, 1.0)
        for g in range(2):
            nc.vector.memset(dcol[g], 1.0)

        # ---------------- phase 3: tab dots + exp ----------------
        # head h key chunk: k_tT feat rows 64h..64h+63 live at
        # partitions (h%2)*64.. of column h//2
        for qb in range(NQB):
            for g in range(2):  # head groups of 4
                ps = half_tile("pstd")
                nc.vector.memset(ps, 0.0)
                for j in range(4):
                    h = 4 * g + j
                    hb = (h % 2) * 64
                    nc.tensor.matmul(
                        ps[32 * j:32 * j + 1, :],
                        lhsT=k_tT[hb:hb + 64, h // 2:h // 2 + 1],
                        rhs=qT[h // 2][hb:hb + 64, qb * QB:(qb + 1) * QB],
                        start=True, stop=True,
                        tile_position=(hb, 32 * j))
                nc.scalar.activation(
                    out=tabexp[0:97, (2 * qb + g) * QB:(2 * qb + g + 1) * QB],
                    in_=ps[0:97, :], func=AF.Exp, scale=SCALE)

        if PHASE_LIMIT <= 3:
            with tc.tile_pool(name="dbg", bufs=2) as dbg_p:
                for t in range(NT):
                    fo = dbg_p.tile([128, DIM], F32, name="dbg_fo", tag="fo")
                    nc.vector.tensor_copy(out=fo, in_=tabexp[:, 0:DIM])
                    nc.sync.dma_start(out=out_d[t * 128:(t + 1) * 128, :], in_=fo)
            return

        # ---------------- phase 4+5: attention, normalize, final (qb-outer) ----
        # Per q-block: all heads' dots/exp/attn@v, then recip+normalize+final
        # for that q-block while the next q-block's attention runs.
        ae_p = ctx.enter_context(tc.tile_pool(name="aep", bufs=2))
        fo_p = ctx.enter_context(tc.tile_pool(name="fout", bufs=2))

        for qb in range(NQB):
            for hp in range(4):
                h0, h1 = 2 * hp, 2 * hp + 1
                ae0 = ae_p.tile([128, 8 * QB], BF16, name="ae0", tag="ae0")
                ae1 = ae_p.tile([128, 8 * QB], BF16, name="ae1", tag="ae1")
                # dots^T + exp, 2 kpos chunks at a time, both heads row-packed
                for kp in range(4):   # kpos chunk pairs
                    ps0 = big_tile("psd0")
                    ps1 = big_tile("psd1")
                    for i, kt in enumerate((2 * kp, 2 * kp + 1)):
                        for hh, ps in ((0, ps0), (1, ps1)):
                            hb = hh * 64
                            nc.tensor.matmul(
                                ps[:, i * QB:(i + 1) * QB],
                                lhsT=kT[hp][hb:hb + 64, kt * 128:(kt + 1) * 128],
                                rhs=qT[hp][hb:hb + 64, qb * QB:(qb + 1) * QB],
                                start=True, stop=True)
                    nc.scalar.activation(
                        out=ae0[:, 2 * kp * QB:(2 * kp + 2) * QB],
                        in_=ps0, func=AF.Exp, scale=SCALE)
                    nc.scalar.activation(
                        out=ae1[:, 2 * kp * QB:(2 * kp + 2) * QB],
                        in_=ps1, func=AF.Exp, scale=SCALE)

                # attn@v with denominator row (M=65)
                for hh, ae in ((0, ae0), (1, ae1)):
                    h = 2 * hp + hh
                    po = o_tile([65, QB], "po")
                    for kt in range(NT):
                        nc.tensor.matmul(
                            po,
                            lhsT=v_sb[kt][:, 65 * h:65 * h + 65],
                            rhs=ae[:, kt * QB:(kt + 1) * QB],
                            start=(kt == 0), stop=False)
                    # tab token contribution (K=1, row strip matching tabexp row)
                    g, j = h // 4, h % 4
                    nc.tensor.matmul(
                        po,
                        lhsT=v_t_aug[32 * j:32 * j + 1, 65 * h:65 * h + 65],
                        rhs=tabexp[32 * j:32 * j + 1,
                                   (2 * qb + g) * QB:(2 * qb + g + 1) * QB],
                        start=False, stop=True,
                        tile_position=(32 * j, 0))
                    # cross-quadrant moves must stay on DVE (bank->quadrant
                    # routing is a DVE feature; ACT semantics unknown)
                    if hh == 0:
                        nc.vector.tensor_copy(
                            out=outuT[hp][0:64, qb * QB:(qb + 1) * QB],
                            in_=po[0:64, :])
                    else:
                        nc.vector.tensor_copy(
                            out=outuT[hp][64:128, qb * QB:(qb + 1) * QB],
                            in_=po[0:64, :])
                    nc.vector.tensor_copy(
                        out=dcol[h // 4][32 * (h % 4):32 * (h % 4) + 1,
                                         qb * QB:(qb + 1) * QB],
                        in_=po[64:65, :])

                # after the 2nd/4th pair, this head-group's denominators are
                # complete: recip -> PE K=1 broadcast matmul -> normalize
                if hp in (1, 3):
                    g = hp // 2
                    qs_ = slice(qb * QB, (qb + 1) * QB)
                    nc.vector.reciprocal_approx_accurate(
                        out=rcol[g][0:97, qs_], in_=dcol[g][0:97, qs_],
                        scratch=rscratch[0:97, 0:QB])
                    nc.vector.tensor_copy(out=rcolb[g][0:97, qs_],
                                          in_=rcol[g][0:97, qs_])
                    # one psum bc tile per pair: rows 0-63 even head, 64-127 odd
                    for pp in range(2):
                        hp_ = 2 * g + pp
                        h0_, h1_ = 2 * hp_, 2 * hp_ + 1
                        pbc = o_tile([128, QB], "pbc")
                        for hh_, h_ in ((0, h0_), (1, h1_)):
                            j_ = 32 * (h_ % 4)
                            nc.tensor.matmul(
                                pbc[64 * hh_:64 * hh_ + 64, :],
                                lhsT=ones128[j_:j_ + 1, 0:64],
                                rhs=rcolb[g][j_:j_ + 1, qs_],
                                start=True, stop=True,
                                tile_position=(j_, 64 * hh_))
                        nc.vector.tensor_tensor(
                            out=outuT[hp_][:, qs_],
                            in0=outuT[hp_][:, qs_],
                            in1=pbc, op=OP.mult)

            # ---- normalize handled per head-group inside the hp loop ----
            qs = slice(qb * QB, (qb + 1) * QB)
            for t in range(4 * qb, 4 * qb + 4):
                pf = half_tile("pf")
                for c in range(NC_):
                    nc.tensor.matmul(
                        pf,
                        lhsT=outuT[c][:, t * 128:(t + 1) * 128],
                        rhs=wo[c],
                        start=(c == 0), stop=(c == NC_ - 1))
                fo = fo_p.tile([128, DIM], F32, name="fo", tag="fo")
                nc.vector.tensor_tensor(out=fo, in0=pf, in1=bout_bc, op=OP.add)
                nc.sync.dma_start(out=out_d[t * 128:(t + 1) * 128, :], in_=fo)


_CACHED_NC = None


def _preprocess(inputs):
    img = np.ascontiguousarray(np.asarray(inputs["img"], dtype=np.float32))
    tab = np.ascontiguousarray(np.asarray(inputs["tab"], dtype=np.float32))
    w_qkv = np.asarray(inputs["w_qkv"], dtype=np.float32)
    w_tab_qkv = np.asarray(inputs["w_tab_qkv"], dtype=np.float32)
    w_out = np.asarray(inputs["w_out"], dtype=np.float32)
    b_out = np.asarray(inputs["b_out"], dtype=np.float32).reshape(1, DIM)
    ln_w = np.asarray(inputs["ln_w"], dtype=np.float32).reshape(DIM)
    ln_b = np.asarray(inputs["ln_b"], dtype=np.float32).reshape(DIM)

    w_tab_kv = w_tab_qkv[:, INNER:3 * INNER]
    # fold ln scale/shift into the weights (LN is linear before the matmul)
    w_qkv_eff = (ln_w[:, None] * w_qkv).astype(NPBF16)
    w_tab_eff = (ln_w[:, None] * w_tab_kv).astype(NPBF16)
    b_qkv = ln_b @ w_qkv          # [1536]
    b_tab = ln_b @ w_tab_kv       # [1024]

    shared = {
        "w_qkv": np.ascontiguousarray(w_qkv_eff),
        "w_tab": np.ascontiguousarray(w_tab_eff),
        "w_out": np.ascontiguousarray(w_out.astype(NPBF16)),
        "b_out": b_out,
        "bqk": np.ascontiguousarray(b_qkv[:2 * INNER].reshape(8, 128).T
                                    .astype(np.float32)),
        "bv": np.ascontiguousarray(b_qkv[2 * INNER:].reshape(1, INNER)
                                   .astype(np.float32)),
        "btabk": np.ascontiguousarray(b_tab[:INNER].reshape(4, 128).T
                                      .astype(np.float32)),
        "btabv": np.ascontiguousarray(b_tab[INNER:].reshape(1, INNER)
                                      .astype(np.float32)),
    }
    in_maps = []
    for b in range(N_CORES):
        m = {"img_s": np.ascontiguousarray(img[b]),
             "tab_s": np.ascontiguousarray(tab[b])}
        m.update(shared)
        in_maps.append(m)
    return in_maps


def make_in_maps(inputs):
    return _preprocess(inputs)


def kernel(**inputs):
    global _CACHED_NC
    if _CACHED_NC is None:
        _CACHED_NC = build_program()
    nc = _CACHED_NC

    in_maps = _preprocess(inputs)
    res = bass_utils.run_bass_kernel_spmd(nc, in_maps, core_ids=list(range(N_CORES)))
    out = np.stack([res.results[c]["out_s"] for c in range(N_CORES)], axis=0)
    return out.astype(np.float32)


if __name__ == "__main__":
    d = np.load("/root/problem/ref_data.npz")
    ins = {k: d[k] for k in ("img", "tab", "w_qkv", "w_tab_qkv", "w_out",
                             "b_out", "ln_w", "ln_b")}
    actual = kernel(**ins)
    expected = d["expected"]
    err = np.abs(actual - expected).max()
    rel = err / np.abs(expected).max()
    print("absmax err:", err, "rel:", rel)
